# revision 43
# baseline (speedup 1.0000x reference)
"""Trainium2 Bass kernel for BasicMGU (nn_BasicMGU_53386443489965).

Math (per reference):
    xz = x @ W_k ; xh = x @ W_u
    f_t = sigmoid(xz_t + h @ W_r + b_r)
    c_t = tanh(xh_t + (h*f_t) @ W_ur + b_ur)
    h   = (1-f_t)*h + f_t*c_t        -> return final h  [B, U]

Sharding: data-parallel over batch across 8 cores (B=64 -> 8 per core),
weights replicated.

Per-core design:
  Phase 1 (projections): two bf16 GEMMs (full PE rate at N=512),
  producing xzT/xhT in DRAM pre-swizzled per-chunk, biases folded in,
  stored BF16 (halves slab DMA; measured rel err 1.07e-2 < 2e-2 gate).
  Chunk slabs are laid out in even/odd tensors with slots shifted by
  one chunk so the recurrence loop can prefetch with a plain ds(it):
  ping-pong SBUF slab tiles are refilled a chunk ahead, overlapping
  the DMA with the other chunk's recurrence (no boundary stall).
  Phase 2 (recurrence): state kept transposed hT [U(part), B(free)].
  Per-step PSUM tiles are initialized with identity-matmuls on the
  tensor engine (I.T @ xz_t) instead of DVE copies - the DVE was ~80%
  busy in the old design and its per-op fixed cost (~170ns) dominated.
  DVE ops run full-width (one op over all 4 m-chunks) where they are
  off the serial critical path; only hf/eb (which gate matmul bursts)
  stay split in m-halves. PSUM halves live in separate banks so
  sigmoid/tanh fire as soon as their half's accumulation completes
  (bank-level dependency tracking; bursts are kh-outer with mh0 first).
  The A/e split hides tanh latency: z1(t+1) = xz(t+1) + A@W_r + e@W_r
  with A = h - h*f and e = f*c (linearity), so the A-part runs during
  the tanh wait.
"""

import os
import sys
import types

sys.path.insert(0, "/opt/trn_rl_repo")

import numpy as np
import ml_dtypes

import concourse.bass as bass
import concourse.mybir as mybir
import concourse.tile as tile
from concourse import bacc
from concourse.bass_utils import run_bass_kernel_spmd

B, T, D, U = 64, 1024, 512, 512
NCORES = 8
BL = B // NCORES          # batch per core
S = int(os.environ.get("MGU_S", 128))  # recurrence steps per hw-loop iteration
KC = D // 128             # contraction chunks
MC = U // 128             # output-unit chunks
MH = MC // 2              # m-chunks per half
PCOLS = 512               # projection (t,b) columns per block
NBLK = T * BL // PCOLS
NW = S * BL               # free width of one swizzled chunk slab

F32 = mybir.dt.float32
F32R = mybir.dt.float32r
BF16 = mybir.dt.bfloat16

LAST_EXEC_NS = None


def _install_trace_shim():
    """Make `antenv.axon_hooks` importable so trace=True degrades gracefully
    (and, where the axon .so is present, actually captures NTFF profiles)."""
    if "antenv.axon_hooks" in sys.modules:
        return
    mod = types.ModuleType("antenv.axon_hooks")
    holder = [None]
    mod.set_axon_ntff_profile_hook = lambda h: holder.__setitem__(0, h)
    mod.get_axon_ntff_profile_hook = lambda: holder[0]
    sys.modules["antenv.axon_hooks"] = mod
    try:
        if "/root/.axon_site" not in sys.path:
            sys.path.append("/root/.axon_site")
        from trn_agent_boot.trn_boot import _ntff_profile_via_ctypes

        hook = _ntff_profile_via_ctypes("/opt/axon/libaxon_pjrt.so")
        if hook is not None:
            mod.set_axon_ntff_profile_hook(hook)
    except Exception:
        pass


def _build():
    nc = bacc.Bacc("TRN2")

    t_total = int(os.environ.get("MGU_TSTEPS", T))
    nch = t_total // S

    xT = nc.dram_tensor("xT", [D, T * BL], BF16, kind="ExternalInput")
    # Shifted per-iteration view of x for in-loop projections: slot k holds
    # blocks 4k+4..4k+7 (chunks 2k+2, 2k+3); the last slot is zeros and its
    # projections land in the garbage ev/od slots (never read).
    nch2_ = int(os.environ.get("MGU_TSTEPS", T)) // S // 2
    xTs = nc.dram_tensor("xTs", [D, nch2_, 4, PCOLS], BF16, kind="ExternalInput")
    Wk = nc.dram_tensor("Wk", [D, U], BF16, kind="ExternalInput")
    Wu = nc.dram_tensor("Wu", [D, U], BF16, kind="ExternalInput")
    Wr = nc.dram_tensor("Wr", [U, U], BF16, kind="ExternalInput")
    Wur = nc.dram_tensor("Wur", [U, U], BF16, kind="ExternalInput")
    br = nc.dram_tensor("br", [U], F32, kind="ExternalInput")
    bur = nc.dram_tensor("bur", [U], F32, kind="ExternalInput")
    Ieye_d = nc.dram_tensor("Ieye", [128, 128], BF16, kind="ExternalInput")
    hT_out = nc.dram_tensor("hT_out", [128, MC, BL], F32, kind="ExternalOutput")
    # Swizzled step-input slabs, bf16. Chunks 0/1 in dedicated prologue
    # tensors; remaining chunks split even/odd with slots SHIFTED so the
    # in-loop prefetch DMA can use a plain ds(it) index:
    #   ev slot k = chunk 2k+2, od slot k = chunk 2k+3 (last slots garbage).
    nch_ = int(os.environ.get("MGU_TSTEPS", T)) // S
    assert nch_ % 2 == 0
    xz_p0 = nc.dram_tensor("xz_p0", [MC, 128, NW], BF16)
    xh_p0 = nc.dram_tensor("xh_p0", [MC, 128, NW], BF16)
    xz_p1 = nc.dram_tensor("xz_p1", [MC, 128, NW], BF16)
    xh_p1 = nc.dram_tensor("xh_p1", [MC, 128, NW], BF16)
    xz_ev = nc.dram_tensor("xz_ev", [nch_ // 2, MC, 128, NW], BF16)
    xh_ev = nc.dram_tensor("xh_ev", [nch_ // 2, MC, 128, NW], BF16)
    xz_od = nc.dram_tensor("xz_od", [nch_ // 2, MC, 128, NW], BF16)
    xh_od = nc.dram_tensor("xh_od", [nch_ // 2, MC, 128, NW], BF16)

    ID = mybir.ActivationFunctionType.Identity
    SIG = mybir.ActivationFunctionType.Sigmoid
    TANH = mybir.ActivationFunctionType.Tanh

    with tile.TileContext(nc) as tc:
        with tc.tile_pool(name="consts", bufs=1) as consts:
            Wk_sb = consts.tile([128, KC, U], BF16)
            nc.sync.dma_start(Wk_sb, Wk[:, :].rearrange("(c p) u -> p c u", p=128))
            Wu_sb = consts.tile([128, KC, U], BF16)
            nc.sync.dma_start(Wu_sb, Wu[:, :].rearrange("(c p) u -> p c u", p=128))
            Wr_sb = consts.tile([128, MC, U], BF16)
            nc.sync.dma_start(Wr_sb, Wr[:, :].rearrange("(c p) u -> p c u", p=128))
            Wur_sb = consts.tile([128, MC, U], BF16)
            nc.sync.dma_start(Wur_sb, Wur[:, :].rearrange("(c p) u -> p c u", p=128))
            br_sb = consts.tile([128, MC], F32)
            nc.sync.dma_start(br_sb, br[:].rearrange("(c p) -> p c", p=128))
            bur_sb = consts.tile([128, MC], F32)
            nc.sync.dma_start(bur_sb, bur[:].rearrange("(c p) -> p c", p=128))

            # Identity stationary for psum-init matmuls (bf16 is exact for 1.0)
            Ieye = consts.tile([128, 128], BF16)
            nc.sync.dma_start(Ieye, Ieye_d[:, :])

            hTf = consts.tile([128, MC, BL], F32)
            nc.vector.memset(hTf, 0.0)
            hTb = consts.tile([128, MC, BL], BF16)
            nc.vector.memset(hTb, 0.0)

            # ---------------- Phase 1: projections ----------------
            with (
                tc.tile_pool(name="proj_in", bufs=2) as pin,
                tc.tile_pool(name="proj_ps", bufs=6, space="PSUM") as pps,
                tc.tile_pool(name="proj_out", bufs=6) as pout,
            ):
                tblk = PCOLS // BL  # timesteps per column block
                assert S % tblk == 0 and S > tblk
                for j in range(2 * (S // tblk)):
                    xT_sb = pin.tile([128, KC, PCOLS], BF16, tag="xT_st")
                    nc.sync.dma_start(
                        xT_sb,
                        xT[:, j * PCOLS : (j + 1) * PCOLS].rearrange(
                            "(c p) n -> p c n", p=128
                        ),
                    )
                    cch = (j * tblk) // S  # chunk this block belongs to
                    jj = j % (S // tblk)  # block position within the chunk
                    cols = slice(jj * PCOLS, (jj + 1) * PCOLS)
                    for W_sb, bias_sb, dsts in (
                        (Wk_sb, br_sb, (xz_p0, xz_p1, xz_ev, xz_od)),
                        (Wu_sb, bur_sb, (xh_p0, xh_p1, xh_ev, xh_od)),
                    ):
                        p0_, p1_, ev_, od_ = dsts
                        for m in range(MC):
                            ps = pps.tile([128, PCOLS], F32)
                            for k in range(KC):
                                nc.tensor.matmul(
                                    ps,
                                    W_sb[:, k, m * 128 : (m + 1) * 128],
                                    xT_sb[:, k, :],
                                    start=(k == 0),
                                    stop=(k == KC - 1),
                                )
                            o = pout.tile([128, PCOLS], BF16)
                            nc.scalar.activation(o, ps, ID, bias=bias_sb[:, m : m + 1])
                            if cch == 0:
                                dst_ap = p0_[m, :, cols]
                            elif cch == 1:
                                dst_ap = p1_[m, :, cols]
                            elif cch % 2 == 0:
                                dst_ap = ev_[(cch - 2) // 2, m, :, cols]
                            else:
                                dst_ap = od_[(cch - 3) // 2, m, :, cols]
                            nc.sync.dma_start(dst_ap, o)

            # ---------------- Phase 2: recurrence ----------------
            with (
                tc.tile_pool(name="rec_in", bufs=1) as rin,
                tc.tile_pool(name="rec_ps1", bufs=2, space="PSUM") as rps1,
                tc.tile_pool(name="rec_ps2", bufs=1, space="PSUM") as rps2,
                tc.tile_pool(name="rec_tmp", bufs=3) as rtmp,
                tc.tile_pool(name="pj_in", bufs=2) as pjin,
                tc.tile_pool(name="pj_ps", bufs=2, space="PSUM") as pjps,
                tc.tile_pool(name="pj_out", bufs=3) as pjout,
            ):
                def mm_bursts(pstiles, W_sb_, rhs_halves, stop_last):
                    # 2x2 burst order: (k-half, m-half) so the first k-burst
                    # starts as soon as rhs half 0 is ready, and each m-half
                    # psum bank completes as early as possible (mh0 first).
                    for kh in range(2):
                        for mh in range(2):
                            for m in range(MH):
                                for k in range(MH):
                                    kk = kh * MH + k
                                    mm = mh * MH + m
                                    nc.tensor.matmul(
                                        pstiles[mh][:, m, :],
                                        W_sb_[:, kk, mm * 128 : (mm + 1) * 128],
                                        rhs_halves[kh][:, k, :],
                                        start=False,
                                        stop=stop_last and kk == KC - 1,
                                    )

                def psum_init(pstiles, src_sb, cols):
                    # init psum halves with identity-matmuls: ps[m] = I.T @ src_m
                    # First MM on each bank carries start=True (clears the bank).
                    for mh in range(2):
                        for m in range(MH):
                            mm = mh * MH + m
                            nc.tensor.matmul(
                                pstiles[mh][:, m, :],
                                Ieye,
                                src_sb[:, mm, cols],
                                start=(m == 0),
                                stop=False,
                            )

                def alloc_ps(pool, tag):
                    return [
                        pool.tile([128, MH, BL], F32, tag=f"{tag}{hh}", name=f"{tag}{hh}")
                        for hh in range(2)
                    ]

                # Ping-pong slab tiles, loaded a chunk ahead so the DMA
                # overlaps the other chunk's recurrence instead of stalling
                # at the loop boundary.
                xzA = rin.tile([128, MC, NW], BF16, tag="xzA")
                xhA = rin.tile([128, MC, NW], BF16, tag="xhA")
                xzB = rin.tile([128, MC, NW], BF16, tag="xzB")
                xhB = rin.tile([128, MC, NW], BF16, tag="xhB")
                nc.sync.dma_start(xzA, xz_p0[:, :, :].rearrange("c p n -> p c n"))
                nc.sync.dma_start(xhA, xh_p0[:, :, :].rearrange("c p n -> p c n"))
                nc.sync.dma_start(xzB, xz_p1[:, :, :].rearrange("c p n -> p c n"))
                nc.sync.dma_start(xhB, xh_p1[:, :, :].rearrange("c p n -> p c n"))

                def proj_items(it, blocks):
                    items = []
                    state = {}

                    def dma_in(bb):
                        def f():
                            xv = pjin.tile([128, KC, PCOLS], BF16, tag="xv", name="xv")
                            state[bb] = xv
                            nc.sync.dma_start(
                                xv,
                                xTs[:, bass.ds(it, 1), bb, :].rearrange(
                                    "(c p) o n -> p c (o n)", p=128
                                ),
                            )
                        return f

                    def mm(bb, W_sb_, m, k):
                        def f():
                            if k == 0:
                                state["ps"] = pjps.tile(
                                    [128, PCOLS], F32, tag="pjps", name="pjps"
                                )
                            nc.tensor.matmul(
                                state["ps"],
                                W_sb_[:, k, m * 128 : (m + 1) * 128],
                                state[bb][:, k, :],
                                start=(k == 0),
                                stop=(k == KC - 1),
                            )
                        return f

                    def actout(bias_sb, dst_t, m, col):
                        def f():
                            o = pjout.tile([128, PCOLS], BF16, tag="pjo", name="pjo")
                            nc.scalar.activation(
                                o, state["ps"], ID, bias=bias_sb[:, m : m + 1]
                            )
                            nc.sync.dma_start(
                                dst_t[
                                    bass.ds(it, 1), m, :, col * PCOLS : (col + 1) * PCOLS
                                ].rearrange("o p n -> p (o n)"),
                                o,
                            )
                        return f

                    for bb, col, zt, ht in blocks:
                        items.append(dma_in(bb))
                        for W_sb_, bias_sb, dst_t in (
                            (Wk_sb, br_sb, zt),
                            (Wu_sb, bur_sb, ht),
                        ):
                            for m in range(MC):
                                for k in range(KC):
                                    items.append(mm(bb, W_sb_, m, k))
                                items.append(actout(bias_sb, dst_t, m, col))
                    return items

                def chunk_body(xz_sb, xh_sb, items=()):
                    # chunk head: step 0's mm1 runs from the bf16 state
                    # snapshot saved at the previous chunk boundary.
                    ps1 = alloc_ps(rps1, "ps1")
                    psum_init(ps1, xz_sb, slice(0, BL))
                    hTb_h = [hTb[:, 0:MH, :], hTb[:, MH:MC, :]]
                    mm_bursts(ps1, Wr_sb, hTb_h, True)
                    for s in range(S):
                        bsl = slice(s * BL, (s + 1) * BL)
                        # psum inits first: they are ready early (banks freed
                        # by last step's sigma/tanh reads) and must not land
                        # mid-burst on the tensor queue where they would delay
                        # the activation gates.
                        ps1n = None
                        if s < S - 1:
                            nsl = slice((s + 1) * BL, (s + 2) * BL)
                            ps1n = alloc_ps(rps1, "ps1")
                            psum_init(ps1n, xz_sb, nsl)
                        ps2 = alloc_ps(rps2, "ps2")
                        psum_init(ps2, xh_sb, bsl)
                        # fT = sigmoid(ps1): per-half so h0 fires as soon as
                        # its psum bank completes.
                        fT = rtmp.tile([128, MC, BL], F32, tag="fT", name="fT")
                        for hh in range(2):
                            msl = slice(hh * MH, (hh + 1) * MH)
                            nc.scalar.activation(fT[:, msl, :], ps1[hh], SIG)
                        # hf halves gate the mm2 k-bursts
                        hfh = rtmp.tile([128, MC, BL], BF16, tag="hf", name="hf")
                        for hh in range(2):
                            msl = slice(hh * MH, (hh + 1) * MH)
                            nc.vector.tensor_mul(
                                hfh[:, msl, :], hTf[:, msl, :], fT[:, msl, :]
                            )
                        # A = h - h*f, full width (off critical path)
                        Ab = rtmp.tile([128, MC, BL], BF16, tag="Ab", name="Ab")
                        nc.vector.tensor_sub(Ab, hTf, hfh)

                        # mm2: c_pre = xh_t + hf @ W_ur
                        hf_h = [hfh[:, 0:MH, :], hfh[:, MH:MC, :]]
                        mm_bursts(ps2, Wur_sb, hf_h, True)

                        # mm1 A-part for next step (overlaps the tanh wait):
                        # z1(t+1) = xz(t+1) + A@W_r + e@W_r  (linearity)
                        if s < S - 1:
                            Ab_h = [Ab[:, 0:MH, :], Ab[:, MH:MC, :]]
                            mm_bursts(ps1n, Wr_sb, Ab_h, False)
                        # drain one projection item mid-step: its matmul runs
                        # in the tanh-wait tensor window (after mm1A), its ACT
                        # copy in the post-sigma ACT window (before the tanhs).
                        if s < len(items):
                            items[s]()

                        cT = rtmp.tile([128, MC, BL], F32, tag="cT", name="cT")
                        for hh in range(2):
                            msl = slice(hh * MH, (hh + 1) * MH)
                            nc.scalar.activation(cT[:, msl, :], ps2[hh], TANH)
                        eb = rtmp.tile([128, MC, BL], BF16, tag="eb", name="eb")
                        for hh in range(2):
                            msl = slice(hh * MH, (hh + 1) * MH)
                            nc.vector.tensor_mul(
                                eb[:, msl, :], cT[:, msl, :], fT[:, msl, :]
                            )
                        if s < S - 1:
                            eb_h = [eb[:, 0:MH, :], eb[:, MH:MC, :]]
                            mm_bursts(ps1n, Wr_sb, eb_h, True)
                        # state update h' = A + e (f32), full width
                        nc.vector.tensor_add(hTf, Ab, eb)
                        if s == S - 1:
                            nc.vector.tensor_add(hTb, Ab, eb)
                        ps1 = ps1n

                with tc.For_i(0, nch // 2, 1, staggered_reset=True) as it:
                    chunk_body(
                        xzA,
                        xhA,
                        proj_items(it, [(0, 0, xz_ev, xh_ev), (1, 1, xz_ev, xh_ev)]),
                    )
                    # refill A with chunk 2it+2 while chunk 2it+1 runs
                    nc.sync.dma_start(
                        xzA,
                        xz_ev[bass.ds(it, 1), :, :, :].rearrange("o c p n -> p (o c) n"),
                    )
                    nc.sync.dma_start(
                        xhA,
                        xh_ev[bass.ds(it, 1), :, :, :].rearrange("o c p n -> p (o c) n"),
                    )
                    chunk_body(
                        xzB,
                        xhB,
                        proj_items(it, [(2, 0, xz_od, xh_od), (3, 1, xz_od, xh_od)]),
                    )
                    # refill B with chunk 2it+3 (lands during the next
                    # iteration's first half)
                    nc.sync.dma_start(
                        xzB,
                        xz_od[bass.ds(it, 1), :, :, :].rearrange("o c p n -> p (o c) n"),
                    )
                    nc.sync.dma_start(
                        xhB,
                        xh_od[bass.ds(it, 1), :, :, :].rearrange("o c p n -> p (o c) n"),
                    )

            nc.sync.dma_start(hT_out[:, :, :], hTf)

    nc.compile()
    return nc


_NC_CACHE = None


def kernel(x, W_k, W_r, b_r, W_u, W_ur, b_ur):
    global _NC_CACHE, LAST_EXEC_NS
    _install_trace_shim()
    if _NC_CACHE is None:
        _NC_CACHE = _build()
    nc = _NC_CACHE

    x = np.ascontiguousarray(np.asarray(x, dtype=np.float32))
    Wr_b = np.asarray(W_r, dtype=np.float32).astype(ml_dtypes.bfloat16)
    Wur_b = np.asarray(W_ur, dtype=np.float32).astype(ml_dtypes.bfloat16)
    Wk_f = np.asarray(W_k, dtype=np.float32).astype(ml_dtypes.bfloat16)
    Wu_f = np.asarray(W_u, dtype=np.float32).astype(ml_dtypes.bfloat16)
    br_f = np.ascontiguousarray(np.asarray(b_r, dtype=np.float32))
    bur_f = np.ascontiguousarray(np.asarray(b_ur, dtype=np.float32))

    in_maps = []
    for c in range(NCORES):
        xc = x[c * BL : (c + 1) * BL]  # [BL, T, D]
        xTc = np.ascontiguousarray(
            xc.transpose(2, 1, 0).reshape(D, T * BL).astype(ml_dtypes.bfloat16)
        )
        nch2 = T // S // 2
        blocks = xTc.reshape(D, NBLK, PCOLS)
        xTs_np = np.zeros((D, nch2, 4, PCOLS), dtype=ml_dtypes.bfloat16)
        for k2 in range(nch2 - 1):
            xTs_np[:, k2] = blocks[:, 4 * k2 + 4 : 4 * k2 + 8]
        in_maps.append(
            {
                "xT": xTc,
                "xTs": np.ascontiguousarray(xTs_np),
                "Wk": Wk_f,
                "Wu": Wu_f,
                "Wr": Wr_b,
                "Wur": Wur_b,
                "br": br_f,
                "bur": bur_f,
                "Ieye": np.eye(128, dtype=ml_dtypes.bfloat16),
            }
        )

    trace = bool(os.environ.get("BASS_TRACE"))
    res = run_bass_kernel_spmd(
        nc, in_maps, core_ids=list(range(NCORES)), trace=trace
    )
    LAST_EXEC_NS = res.exec_time_ns

    out = np.empty((B, U), dtype=np.float32)
    for c in range(NCORES):
        hT = res.results[c]["hT_out"]  # [128, MC, BL]
        out[c * BL : (c + 1) * BL] = hT.transpose(2, 1, 0).reshape(BL, U)
    return out
